# revision 7
# baseline (speedup 1.0000x reference)
"""Depthwise Conv1d (C=512, K=3, stride=1, pad=1) on 8 Trainium2 NeuronCores.

Problem: x [16, 512, 4096] f32, w [512, 1, 3] f32, b [512] f32
         out[n,c,l] = sum_k w[c,0,k] * x_pad[n,c,l+k] + b[c]

Correctness gate is rel_err < 2e-2; fp16 I/O keeps the L2 rel err at
~3.6e-4 while halving HBM traffic.

Sharding: (channel-block, batch-group) — core c handles channel block
c%4 (128 channels) for 8 of the 16 batches, i.e. 8 row-tiles of
[128, 4096].

Timing model (measured via perfetto): engines come up ~5.9 us (fixed
NEFF entry handshake), a cold DGE doorbell takes ~2.7 us to first
bytes, the NEFF exit handshake is ~3.2 us. The middle is jointly
bound by the PE stream (192 N=512 matmuls = 41.7 us) and the HBM
window (16.8 MB at ~358 GB/s = 47 us). The schedule packs both ends:
  - x in DRAM is UNPADDED [NT, 128, 4096]: every full-tile load line
    is a clean 8 KB per partition (a 4098-col padded layout crosses
    the 8 KB descriptor boundary, ~30% worse load queue efficiency).
    Conv padding lives in SBUF: payload at columns 16..4112 of a
    [128, 4114] tile; columns 15 and 4112 are zeroed once per buffer.
  - ALL loads issued up front on the sync ring (xpool bufs=8 -> no
    recycling deps; descriptors flood the queues early so loads win
    bandwidth; PE streams with zero data stalls once running). SBUF
    holds the whole working set (~17 MB).
  - tile 0 loads in fine slices, tile 1 in halves (early DMA runs at
    reduced ramp bandwidth; fine pieces keep the PE fed), tiles 2..7
    as single 1.05 MB transfers.
  - weights + bias ride the gpsimd ring (fires with the cold-start
    latency, landing ~8.9 us — just before the warmup matmuls end)
  - warmup matmuls on a gpsimd-memset scratch tile ramp the HAM clock
    gate (4/8 -> 8/8) from ~6.6 us so the real stream starts warm the
    moment tile 0's first slice lands (~9.9 us)
  - conv on TensorE: per 512-col chunk, 3 matmuls with stationary
    diag(w_k) [128,128] fp16, accumulated into one PSUM bank (fp32)
  - evict PSUM -> SBUF fp16 with per-partition bias add, alternating
    VectorE tensor_scalar / ScalarE activation
  - stores alternate the gpsimd and scalar rings (one ring alone is
    ~270 GB/s; two keep stores off the critical path and both rings
    warm so late descriptors start promptly). The last tile stores
    per-chunk; its final chunk's eviction is split across both
    engines and stored as two halves on the two warm rings to
    minimize the post-PE tail.
"""

import numpy as np

B, C, L, K = 16, 512, 4096, 3
N_CORES = 8
NBLK = 4                     # channel blocks of 128
B_SH = 8                     # batches per core
NT = B_SH                    # row-tiles per core (one channel block each)
CH = 512                     # matmul chunk columns (1 PSUM bank; fp16 ISA max)
NCH = L // CH
HALF = L // 2
OFF = 16                     # payload start col in SBUF (32 B aligned)
XW = OFF + L + 2             # SBUF x-tile width: pads at OFF-1 and OFF+L

_STATE = {}


def _build_program():
    from contextlib import ExitStack

    import concourse.bacc as bacc
    import concourse.mybir as mybir
    import concourse.tile as tile

    f16 = mybir.dt.float16
    f32 = mybir.dt.float32
    nc = bacc.Bacc(
        "TRN2",
        target_bir_lowering=False,
        debug=False,
        num_devices=N_CORES,
    )
    x_d = nc.dram_tensor("x", [NT, 128, L], f16, kind="ExternalInput").ap()
    wd_d = nc.dram_tensor("wd", [128, 3 * 128], f16, kind="ExternalInput").ap()
    bias_d = nc.dram_tensor("bias", [128, 1], f32, kind="ExternalInput").ap()
    o_d = nc.dram_tensor("out", [NT, 128, L], f16, kind="ExternalOutput").ap()

    with tile.TileContext(nc) as tc, ExitStack() as ctx:
        wpool = ctx.enter_context(tc.tile_pool(name="wpool", bufs=1))
        xpool = ctx.enter_context(tc.tile_pool(name="xpool", bufs=NT))
        opool = ctx.enter_context(tc.tile_pool(name="opool", bufs=NT))
        ppool = ctx.enter_context(
            tc.tile_pool(name="ppool", bufs=5, space="PSUM")
        )

        wd = wpool.tile([128, 3 * 128], f16)
        bias = wpool.tile([128, 1], f32)
        nc.gpsimd.dma_start(wd[:, :], wd_d)
        nc.gpsimd.dma_start(bias[:, :], bias_d)

        # scratch for PE warmup — the FIRST vector instruction (gpsimd
        # memset pays a one-time ucode load, ~3 us; vector does not)
        warm = wpool.tile([128, CH], f16)
        nc.vector.memset(warm[:, :], 0.0)

        # All loads up front. Tile 0 finely sliced, tile 1 in halves,
        # tiles 2..7 single transfers with clean 8 KB lines. Pad columns
        # zeroed once per buffer (payload never touches them).
        xps = []
        for t in range(NT):
            xp = xpool.tile([128, XW], f16, tag="xp")
            nc.vector.memset(xp[:, OFF - 1 : OFF], 0.0)
            nc.vector.memset(xp[:, OFF + L : OFF + L + 1], 0.0)
            if t == 0:
                cuts = [0, 530, 1090, 2114, 3138, L]
            elif t == 1:
                cuts = [0, 2050, L]
            else:
                cuts = [0, L]
            for a, b in zip(cuts, cuts[1:]):
                nc.sync.dma_start(xp[:, OFF + a : OFF + b], x_d[t][:, a:b])
            xps.append(xp)

        # dummy matmuls warm the HAM clock gate (4/8 -> 8/8) so the real
        # stream runs at 2.4 GHz from its first group
        wps = ppool.tile([128, CH], f32, tag="warm", bufs=1)
        for _ in range(5):
            nc.tensor.matmul(
                wps[:, :], warm[:, 0:128], warm[:, :], start=True, stop=True
            )

        for t in range(NT):
            xp = xps[t]
            last = t == NT - 1
            ot = opool.tile([128, L], f16, tag="ot")
            for c in range(NCH):
                if last and c == NCH - 1:
                    # final chunk as two 256-col PSUM chunks so the tail
                    # eviction+store pipeline starts half a chunk earlier;
                    # each half evicts on its own engine and stores
                    # immediately (scalar self-triggers its ring)
                    for h in range(2):
                        lo = c * CH + h * (CH // 2)
                        hi = lo + CH // 2
                        oc = ot[:, lo:hi]
                        ps = ppool.tile([128, CH // 2], f32, tag="psh", bufs=2)
                        base = OFF - 1 + lo
                        for k in range(3):
                            nc.tensor.matmul(
                                ps[:, :],
                                wd[:, k * 128 : (k + 1) * 128],
                                xp[:, base + k : base + k + CH // 2],
                                start=(k == 0),
                                stop=(k == 2),
                            )
                        if h == 0:
                            nc.vector.tensor_scalar(
                                oc, ps[:, :], bias[:, 0:1], None,
                                mybir.AluOpType.add,
                            )
                            nc.gpsimd.dma_start(o_d[t][:, lo:hi], ot[:, lo:hi])
                        else:
                            nc.scalar.activation(
                                oc,
                                ps[:, :],
                                mybir.ActivationFunctionType.Identity,
                                bias=bias[:, 0:1],
                                scale=1.0,
                            )
                            nc.scalar.dma_start(o_d[t][:, lo:hi], ot[:, lo:hi])
                    continue
                oc = ot[:, c * CH : (c + 1) * CH]
                ps = ppool.tile([128, CH], f32, tag="ps")
                base = OFF - 1 + c * CH
                for k in range(3):
                    nc.tensor.matmul(
                        ps[:, :],
                        wd[:, k * 128 : (k + 1) * 128],
                        xp[:, base + k : base + k + CH],
                        start=(k == 0),
                        stop=(k == 2),
                    )
                if c % 2 == 0:
                    nc.vector.tensor_scalar(
                        oc, ps[:, :], bias[:, 0:1], None, mybir.AluOpType.add
                    )
                else:
                    nc.scalar.activation(
                        oc,
                        ps[:, :],
                        mybir.ActivationFunctionType.Identity,
                        bias=bias[:, 0:1],
                        scale=1.0,
                    )
                if not last:
                    continue
                # last row-tile: fine-grained stores on alternating warm
                # rings to shorten the tail
                if c == 3:
                    nc.gpsimd.dma_start(o_d[t][:, 0:HALF], ot[:, 0:HALF])
                elif c == 5:
                    nc.scalar.dma_start(
                        o_d[t][:, HALF : 6 * CH], ot[:, HALF : 6 * CH]
                    )
                elif c == 6:
                    nc.gpsimd.dma_start(
                        o_d[t][:, 6 * CH : 7 * CH], ot[:, 6 * CH : 7 * CH]
                    )
            if not last:
                if t % 2 == 0:
                    nc.gpsimd.dma_start(o_d[t], ot[:, :])
                else:
                    nc.scalar.dma_start(o_d[t], ot[:, :])

    nc.compile()
    return nc


def _pack_weights(w, b):
    """Per channel block: 3 diag [128,128] f16 stacked -> [128, 384], plus
    the f32 bias column [128, 1]."""
    w = np.asarray(w, dtype=np.float32).reshape(C, K)
    b = np.asarray(b, dtype=np.float32)
    wds, biases = [], []
    idx = np.arange(128)
    for blk in range(NBLK):
        wblk = w[blk * 128 : (blk + 1) * 128]
        wd = np.zeros((128, 3 * 128), np.float16)
        for k in range(3):
            wd[idx, k * 128 + idx] = wblk[:, k].astype(np.float16)
        wds.append(wd)
        biases.append(b[blk * 128 : (blk + 1) * 128].reshape(128, 1))
    return wds, biases


def _run(inputs, trace=False, **kw):
    from concourse.bass_utils import run_bass_kernel_spmd

    if "nc" not in _STATE:
        _STATE["nc"] = _build_program()
    nc = _STATE["nc"]

    x = np.asarray(inputs["x"], dtype=np.float32).astype(np.float16)
    wds, biases = _pack_weights(inputs["w"], inputs["b"])
    in_maps = []
    for core in range(N_CORES):
        blk = core % NBLK
        g = core // NBLK
        shard = np.ascontiguousarray(
            x[g * B_SH : (g + 1) * B_SH, blk * 128 : (blk + 1) * 128, :]
        )
        in_maps.append({"x": shard, "wd": wds[blk], "bias": biases[blk]})
    res = run_bass_kernel_spmd(
        nc, in_maps, core_ids=list(range(N_CORES)), trace=trace, **kw
    )
    out = np.empty((B, C, L), np.float32)
    for core in range(N_CORES):
        blk = core % NBLK
        g = core // NBLK
        out[g * B_SH : (g + 1) * B_SH, blk * 128 : (blk + 1) * 128, :] = res.results[
            core
        ]["out"].astype(np.float32)
    return out, res


def kernel(**inputs):
    return _run(inputs)[0]


# revision 8
# speedup vs baseline: 1.0277x; 1.0277x over previous
"""Depthwise Conv1d (C=512, K=3, stride=1, pad=1) on 8 Trainium2 NeuronCores.

Problem: x [16, 512, 4096] f32, w [512, 1, 3] f32, b [512] f32
         out[n,c,l] = sum_k w[c,0,k] * x_pad[n,c,l+k] + b[c]

Correctness gate is rel_err < 2e-2; fp16 I/O keeps the L2 rel err at
~3.6e-4 while halving HBM traffic.

Sharding: (channel-block, batch-group) — core c handles channel block
c%4 (128 channels) for 8 of the 16 batches, i.e. 8 row-tiles of
[128, 4096].

Timing model (measured via perfetto): engines come up ~5.9 us (fixed
NEFF entry handshake), a cold DGE doorbell takes ~2.7 us to first
bytes, the NEFF exit handshake is ~3.2 us. The middle is jointly
bound by the PE stream (192 N=512 matmuls = 41.7 us) and the HBM
window (16.8 MB at ~358 GB/s = 47 us). The schedule packs both ends:
  - x in DRAM is UNPADDED [NT, 128, 4096]: every full-tile load line
    is a clean 8 KB per partition (a 4098-col padded layout crosses
    the 8 KB descriptor boundary, ~30% worse load queue efficiency).
    Conv padding lives in SBUF: payload at columns 16..4112 of a
    [128, 4114] tile; columns 15 and 4112 are zeroed once per buffer.
  - ALL loads issued up front on the sync ring (xpool bufs=8 -> no
    recycling deps; descriptors flood the queues early so loads win
    bandwidth; PE streams with zero data stalls once running). SBUF
    holds the whole working set (~17 MB).
  - tile 0 loads in fine slices, tile 1 in halves (early DMA runs at
    reduced ramp bandwidth; fine pieces keep the PE fed), tiles 2..7
    as single 1.05 MB transfers.
  - weights + bias ride the gpsimd ring (fires with the cold-start
    latency, landing ~8.9 us — just before the warmup matmuls end)
  - warmup matmuls on a gpsimd-memset scratch tile ramp the HAM clock
    gate (4/8 -> 8/8) from ~6.6 us so the real stream starts warm the
    moment tile 0's first slice lands (~9.9 us)
  - conv on TensorE: per 512-col chunk, 3 matmuls with stationary
    diag(w_k) [128,128] fp16, accumulated into one PSUM bank (fp32)
  - evict PSUM -> SBUF fp16 with per-partition bias add, alternating
    VectorE tensor_scalar / ScalarE activation
  - stores alternate the gpsimd and scalar rings (one ring alone is
    ~270 GB/s; two keep stores off the critical path and both rings
    warm so late descriptors start promptly). The last tile stores
    per-chunk; its final chunk's eviction is split across both
    engines and stored as two halves on the two warm rings to
    minimize the post-PE tail.
"""

import numpy as np

B, C, L, K = 16, 512, 4096, 3
N_CORES = 8
NBLK = 4                     # channel blocks of 128
B_SH = 8                     # batches per core
NT = B_SH                    # row-tiles per core (one channel block each)
CH = 512                     # matmul chunk columns (1 PSUM bank; fp16 ISA max)
NCH = L // CH
HALF = L // 2
OFF = 16                     # payload start col in SBUF (32 B aligned)
XW = OFF + L + 2             # SBUF x-tile width: pads at OFF-1 and OFF+L

_STATE = {}


def _build_program():
    from contextlib import ExitStack

    import concourse.bacc as bacc
    import concourse.mybir as mybir
    import concourse.tile as tile

    f16 = mybir.dt.float16
    f32 = mybir.dt.float32
    nc = bacc.Bacc(
        "TRN2",
        target_bir_lowering=False,
        debug=False,
        num_devices=N_CORES,
    )
    x_d = nc.dram_tensor("x", [NT, 128, L], f16, kind="ExternalInput").ap()
    wd_d = nc.dram_tensor("wd", [128, 3 * 128], f16, kind="ExternalInput").ap()
    bias_d = nc.dram_tensor("bias", [128, 1], f32, kind="ExternalInput").ap()
    o_d = nc.dram_tensor("out", [NT, 128, L], f16, kind="ExternalOutput").ap()

    with tile.TileContext(nc) as tc, ExitStack() as ctx:
        wpool = ctx.enter_context(tc.tile_pool(name="wpool", bufs=1))
        xpool = ctx.enter_context(tc.tile_pool(name="xpool", bufs=NT))
        opool = ctx.enter_context(tc.tile_pool(name="opool", bufs=NT))
        ppool = ctx.enter_context(
            tc.tile_pool(name="ppool", bufs=5, space="PSUM")
        )

        wd = wpool.tile([128, 3 * 128], f16)
        bias = wpool.tile([128, 1], f32)
        # weights lead the sync ring: the gpsimd ring cold-starts too
        # slowly (~13 us to land wd there, gating the first real matmul)
        nc.sync.dma_start(wd[:, :], wd_d)
        nc.sync.dma_start(bias[:, :], bias_d)

        # scratch for PE warmup — the FIRST vector instruction (gpsimd
        # memset pays a one-time ucode load, ~3 us; vector does not)
        warm = wpool.tile([128, CH], f16)
        nc.vector.memset(warm[:, :], 0.0)

        # All loads up front. Tile 0 finely sliced, tile 1 in halves,
        # tiles 2..7 single transfers with clean 8 KB lines. Pad columns
        # zeroed once per buffer (payload never touches them).
        xps = []
        for t in range(NT):
            xp = xpool.tile([128, XW], f16, tag="xp")
            nc.vector.memset(xp[:, OFF - 1 : OFF], 0.0)
            nc.vector.memset(xp[:, OFF + L : OFF + L + 1], 0.0)
            if t == 0:
                cuts = [0, 530, 1090, 2114, 3138, L]
            elif t == 1:
                cuts = [0, 2050, L]
            else:
                cuts = [0, L]
            for a, b in zip(cuts, cuts[1:]):
                nc.sync.dma_start(xp[:, OFF + a : OFF + b], x_d[t][:, a:b])
            xps.append(xp)

        # dummy matmuls warm the HAM clock gate (4/8 -> 8/8) so the real
        # stream runs at 2.4 GHz from its first group
        wps = ppool.tile([128, CH], f32, tag="warm", bufs=1)
        for _ in range(6):
            nc.tensor.matmul(
                wps[:, :], warm[:, 0:128], warm[:, :], start=True, stop=True
            )

        for t in range(NT):
            xp = xps[t]
            last = t == NT - 1
            ot = opool.tile([128, L], f16, tag="ot")
            for c in range(NCH):
                if last and c == NCH - 1:
                    # final chunk as two 256-col PSUM chunks so the tail
                    # eviction+store pipeline starts half a chunk earlier;
                    # each half evicts on its own engine and stores
                    # immediately (scalar self-triggers its ring)
                    for h in range(2):
                        lo = c * CH + h * (CH // 2)
                        hi = lo + CH // 2
                        oc = ot[:, lo:hi]
                        ps = ppool.tile([128, CH // 2], f32, tag="psh", bufs=2)
                        base = OFF - 1 + lo
                        for k in range(3):
                            nc.tensor.matmul(
                                ps[:, :],
                                wd[:, k * 128 : (k + 1) * 128],
                                xp[:, base + k : base + k + CH // 2],
                                start=(k == 0),
                                stop=(k == 2),
                            )
                        if h == 0:
                            nc.vector.tensor_scalar(
                                oc, ps[:, :], bias[:, 0:1], None,
                                mybir.AluOpType.add,
                            )
                            nc.gpsimd.dma_start(o_d[t][:, lo:hi], ot[:, lo:hi])
                        else:
                            nc.scalar.activation(
                                oc,
                                ps[:, :],
                                mybir.ActivationFunctionType.Identity,
                                bias=bias[:, 0:1],
                                scale=1.0,
                            )
                            nc.scalar.dma_start(o_d[t][:, lo:hi], ot[:, lo:hi])
                    continue
                oc = ot[:, c * CH : (c + 1) * CH]
                ps = ppool.tile([128, CH], f32, tag="ps")
                base = OFF - 1 + c * CH
                for k in range(3):
                    nc.tensor.matmul(
                        ps[:, :],
                        wd[:, k * 128 : (k + 1) * 128],
                        xp[:, base + k : base + k + CH],
                        start=(k == 0),
                        stop=(k == 2),
                    )
                if c % 2 == 0:
                    nc.vector.tensor_scalar(
                        oc, ps[:, :], bias[:, 0:1], None, mybir.AluOpType.add
                    )
                else:
                    nc.scalar.activation(
                        oc,
                        ps[:, :],
                        mybir.ActivationFunctionType.Identity,
                        bias=bias[:, 0:1],
                        scale=1.0,
                    )
                if not last:
                    continue
                # last row-tile: fine-grained stores on alternating warm
                # rings to shorten the tail
                if c == 3:
                    nc.gpsimd.dma_start(o_d[t][:, 0:HALF], ot[:, 0:HALF])
                elif c == 5:
                    nc.scalar.dma_start(
                        o_d[t][:, HALF : 6 * CH], ot[:, HALF : 6 * CH]
                    )
                elif c == 6:
                    nc.gpsimd.dma_start(
                        o_d[t][:, 6 * CH : 7 * CH], ot[:, 6 * CH : 7 * CH]
                    )
            if not last:
                if t % 2 == 0:
                    nc.gpsimd.dma_start(o_d[t], ot[:, :])
                else:
                    nc.scalar.dma_start(o_d[t], ot[:, :])

    nc.compile()
    return nc


def _pack_weights(w, b):
    """Per channel block: 3 diag [128,128] f16 stacked -> [128, 384], plus
    the f32 bias column [128, 1]."""
    w = np.asarray(w, dtype=np.float32).reshape(C, K)
    b = np.asarray(b, dtype=np.float32)
    wds, biases = [], []
    idx = np.arange(128)
    for blk in range(NBLK):
        wblk = w[blk * 128 : (blk + 1) * 128]
        wd = np.zeros((128, 3 * 128), np.float16)
        for k in range(3):
            wd[idx, k * 128 + idx] = wblk[:, k].astype(np.float16)
        wds.append(wd)
        biases.append(b[blk * 128 : (blk + 1) * 128].reshape(128, 1))
    return wds, biases


def _run(inputs, trace=False, **kw):
    from concourse.bass_utils import run_bass_kernel_spmd

    if "nc" not in _STATE:
        _STATE["nc"] = _build_program()
    nc = _STATE["nc"]

    x = np.asarray(inputs["x"], dtype=np.float32).astype(np.float16)
    wds, biases = _pack_weights(inputs["w"], inputs["b"])
    in_maps = []
    for core in range(N_CORES):
        blk = core % NBLK
        g = core // NBLK
        shard = np.ascontiguousarray(
            x[g * B_SH : (g + 1) * B_SH, blk * 128 : (blk + 1) * 128, :]
        )
        in_maps.append({"x": shard, "wd": wds[blk], "bias": biases[blk]})
    res = run_bass_kernel_spmd(
        nc, in_maps, core_ids=list(range(N_CORES)), trace=trace, **kw
    )
    out = np.empty((B, C, L), np.float32)
    for core in range(N_CORES):
        blk = core % NBLK
        g = core // NBLK
        out[g * B_SH : (g + 1) * B_SH, blk * 128 : (blk + 1) * 128, :] = res.results[
            core
        ]["out"].astype(np.float32)
    return out, res


def kernel(**inputs):
    return _run(inputs)[0]
